# revision 6
# baseline (speedup 1.0000x reference)
"""Trainium2 Bass kernel for batched scaled-dot-product attention.

Problem (all fp32):
    q = queries @ Wq + bq          [B=4, N=4096, E=64]   (D_MODEL=768)
    k = keys    @ Wk + bk
    v = values  @ Wv + bv
    out = softmax(q k^T / sqrt(E)) @ v                    [B, N, 64]

Sharding: 8 cores, data-parallel over batch x query-half.  Core c handles
batch b=c//2, query rows [h*2048, (h+1)*2048) with h=c%2; it loads the full
keys/values for its batch (softmax needs every key).

v2 design (vs the 172us f32r baseline):
  * Inputs converted to fp16 on the host, staged pre-transposed as
    [128, 6, seq] (feature-major): halves HBM traffic to ~16.5MB/core while
    fp16's 10 mantissa bits keep the end-to-end error at ~5e-4.
  * Everything on-chip is fp16 (same full PE rate and same SBUF/DVE cost
    as bf16): qT [64,2048], kT [64,4096], va [128,32,66] (v natural + two
    ones columns so attention-weight row sums fall out of the AV matmul).
  * Two passes of two 512-query blocks (PSUM: 2 oT banks + S-tile pairs +
    proj + transpose bank = 8).  Per k-tile: S^T [128,1024] via two
    matmuls into one 2-bank PSUM tile, ONE wide exp on the scalar engine
    (W=1024 amortizes the ~300-cycle ACTIVATE overhead), then two AV
    matmuls accumulate oT [66,512] per block.
  * AV is issued 2 k-tiles behind S so the PE never stalls on the
    exp latency (v1 lost ~900ns per k-tile to this).
  * PE warm-up: a burst of fp32 dummy matmuls at t=0 lifts the HAM clock
    gate to 2.4GHz before real work arrives (v1 ran its first 45us at
    1.2GHz).
  * Weights/biases packed into single DMAs; x streamed in 1024-col chunks
    with the k/v projections of chunk c+1 hand-interleaved between the
    attention k-tiles of chunk c.
"""

import numpy as np

B, N, D, E = 4, 4096, 768, 64
NCORES = 8
HALF = N // 2          # query rows per core
CH = D // 128          # 6 feature chunks of the contraction dim
KT = N // 128          # 32 key tiles
BLK = 512              # query block (one PSUM bank of fp32)
CHUNK = 1024           # DMA / projection chunk (seq cols)
NCHUNK = N // CHUNK    # 4
SCALE = 1.0 / 8.0      # 1/sqrt(E)
MA = E + 2             # va stationary width (v + two ones columns)
WARMUP_MMS = 16        # fp32 dummy matmuls to lift the HAM clock gate

_CACHE = {}


def _build():
    from contextlib import ExitStack

    import concourse.mybir as mybir
    import concourse.tile as tile
    from concourse import bacc
    from concourse.masks import make_identity

    f32 = mybir.dt.float32
    f32r = mybir.dt.float32r
    f16 = mybir.dt.float16
    EXP = mybir.ActivationFunctionType.Exp

    nc = bacc.Bacc(trn_type="TRN2")
    # host-prepared, fp16, feature-major [128, CH, seq]
    x_q = nc.dram_tensor("x_q", [128, CH, HALF], f16, kind="ExternalInput")
    x_k = nc.dram_tensor("x_k", [128, CH, N], f16, kind="ExternalInput")
    x_v = nc.dram_tensor("x_v", [128, CH, N], f16, kind="ExternalInput")
    # host-prepared: all three weights in one tensor [128, 3, CH, E] (q,k,v)
    w_all = nc.dram_tensor("w_all", [128, 3, CH, E], f16, kind="ExternalInput")
    b_all = nc.dram_tensor("b_all", [E, 3], f32, kind="ExternalInput")
    out = nc.dram_tensor("out", [HALF, E], f32, kind="ExternalOutput")

    with tile.TileContext(nc) as tc, ExitStack() as ctx:
        singles = ctx.enter_context(tc.tile_pool(name="singles", bufs=1))

        ident = singles.tile([128, 128], f32)
        make_identity(nc, ident)
        ident_r = singles.tile([128, 128], f32r)
        nc.vector.tensor_copy(ident_r, ident)
        ident_h = singles.tile([128, 128], f16)
        nc.vector.tensor_copy(ident_h, ident)

        # ---- input DMAs, issued in consumption order ----
        xq_s, xk_s, xv_s = [], [], []

        def stage(x_dr, i, lst, nm):
            t = xs_pool.tile([128, CH, CHUNK], f16, tag="xT", name=nm)
            nc.sync.dma_start(out=t, in_=x_dr[:, :, i * CHUNK:(i + 1) * CHUNK])
            lst.append(t)

        xs_pool = ctx.enter_context(tc.tile_pool(name="xs", bufs=4))
        w_sb = singles.tile([128, 3, CH, E], f16)
        b_sb = singles.tile([E, 3], f32)
        stage(x_q, 0, xq_s, "xq0")
        nc.sync.dma_start(out=w_sb, in_=w_all[:, :, :, :])
        nc.sync.dma_start(out=b_sb, in_=b_all[:, :])
        stage(x_k, 0, xk_s, "xk0")
        stage(x_v, 0, xv_s, "xv0")
        stage(x_q, 1, xq_s, "xq1")
        for i in range(1, NCHUNK):
            stage(x_k, i, xk_s, f"xk{i}")
            stage(x_v, i, xv_s, f"xv{i}")

        bqs_sb = singles.tile([E, 1], f32)
        nc.scalar.mul(bqs_sb, b_sb[:, 0:1], SCALE)  # bq / sqrt(E)

        qT = singles.tile([E, HALF], f16)       # q^T / sqrt(E)
        kT = singles.tile([E, N], f16)          # k^T
        va = singles.tile([128, KT, MA], f16)   # v natural + two ones columns
        nc.vector.memset(va[:, :, E:], 1.0)

        # preload the Exp table off the critical path
        dummy = singles.tile([128, 1], f32)
        nc.scalar.activation(dummy, ident[:, 0:1], EXP)

        vT_pool = ctx.enter_context(tc.tile_pool(name="vT", bufs=2))
        pT_pool = ctx.enter_context(tc.tile_pool(name="pT", bufs=4))
        ep_pool = ctx.enter_context(tc.tile_pool(name="ep", bufs=2))

        def proj(xs, sub, w_idx, pool, dst, dst_col, scale, bias):
            """Project one 512-col subgroup of a staged x chunk into dst."""
            ps = pool.tile([E, BLK], f32, tag="pj", name="ps")
            for c in range(CH):
                nc.tensor.matmul(
                    ps, lhsT=w_sb[:, w_idx, c, :],
                    rhs=xs[:, c, sub * BLK:(sub + 1) * BLK],
                    start=(c == 0), stop=(c == CH - 1))
            if scale is None:
                nc.vector.tensor_scalar(
                    dst[:, dst_col:dst_col + BLK], ps, bias, None,
                    mybir.AluOpType.add)
            else:
                nc.vector.tensor_scalar(
                    dst[:, dst_col:dst_col + BLK], ps, scale, bias,
                    mybir.AluOpType.mult, mybir.AluOpType.add)

        vT_tiles = {}

        def va_chunk(aux_pool, kt):
            """Transpose one 128-col slice of a vT chunk into va[:, kt]."""
            vT_c = vT_tiles[kt // (CHUNK // 128)]
            j = kt % (CHUNK // 128)
            po = aux_pool.tile([128, 128], f16, tag="po", name="po")
            nc.tensor.transpose(
                po[:, :E], vT_c[:, j * 128:(j + 1) * 128], ident_h[:E, :E])
            nc.vector.tensor_copy(va[:, kt, 0:E], po[:, :E])

        def s_exp(s_pool, kt, blk_lo):
            """S^T for k-tile kt, query blocks (blk_lo, blk_lo+1) + wide exp."""
            s2 = s_pool.tile([128, 2 * BLK], f32, tag="s", name="s2")
            for i in range(2):
                nc.tensor.matmul(
                    s2[:, i * BLK:(i + 1) * BLK],
                    lhsT=kT[:, kt * 128:(kt + 1) * 128],
                    rhs=qT[:, (blk_lo + i) * BLK:(blk_lo + i + 1) * BLK],
                    start=True, stop=True, skip_group_check=True)
            pT2 = pT_pool.tile([128, 2 * BLK], f16, tag="pT")
            nc.scalar.activation(pT2, s2, EXP)
            return pT2

        def av(kt, pT2, oT, first, last):
            for i in range(2):
                nc.tensor.matmul(
                    oT[i],
                    lhsT=va[:, kt, :],
                    rhs=pT2[:, i * BLK:(i + 1) * BLK],
                    start=first, stop=last, skip_group_check=True)

        def epilogue(aux_pool, blk, oT_blk):
            oT_sb = ep_pool.tile([MA, BLK], f32r, tag="oT_sb")
            nc.scalar.copy(oT_sb, oT_blk)
            obuf = ep_pool.tile([128, 4, E], f32, tag="obuf")
            for j in range(4):
                op = aux_pool.tile([128, 128], f32r, tag="po", name="op")
                nc.tensor.transpose(
                    op[:, :MA], oT_sb[:, j * 128:(j + 1) * 128],
                    ident_r[:MA, :MA])
                o_sb = ep_pool.tile([128, MA], f32, tag="o_sb")
                nc.vector.tensor_copy(o_sb, op[:, :MA])
                rec = ep_pool.tile([128, 1], f32, tag="rec")
                nc.vector.reciprocal(rec, o_sb[:, E:E + 1])
                nc.vector.tensor_scalar_mul(obuf[:, j, :], o_sb[:, 0:E], rec)
            nc.sync.dma_start(
                out=out[blk * BLK:(blk + 1) * BLK, :].rearrange(
                    "(j p) e -> p j e", p=128),
                in_=obuf)

        def attention_pass(s_pool, aux_pool, pj_pool, blk_lo, tasks):
            """One sweep over all 32 k-tiles for query blocks blk_lo..+1.

            tasks: list of (kt_slot, fn) extra work to interleave."""
            oT = [o_cur.tile([MA, BLK], f32, tag=f"oT{blk_lo + i}",
                             name=f"oT{blk_lo + i}") for i in range(2)]
            pend = {}
            for kt, fn in tasks:
                pend.setdefault(kt, []).append(fn)
            pT_hist = {}
            for kt in range(KT):
                if aux_pool is not None and blk_lo == 0:
                    va_chunk(aux_pool, kt)
                pT_hist[kt] = s_exp(s_pool, kt, blk_lo)
                for fn in pend.pop(kt, ()):
                    fn()
                if kt >= 2:
                    av(kt - 2, pT_hist.pop(kt - 2), oT,
                       first=(kt - 2 == 0), last=False)
            av(KT - 2, pT_hist.pop(KT - 2), oT, first=False, last=False)
            av(KT - 1, pT_hist.pop(KT - 1), oT, first=False, last=True)
            return oT

        # ================= prologue =================
        from contextlib import ExitStack as _ES

        with _ES() as pro:
            warm_ps = pro.enter_context(
                tc.tile_pool(name="warm", bufs=1, space="PSUM"))
            pjq = pro.enter_context(
                tc.tile_pool(name="pjq", bufs=2, space="PSUM"))
            wp = warm_ps.tile([128, 128], f32, tag="w", name="wp")
            for _ in range(WARMUP_MMS):
                nc.tensor.matmul(wp, lhsT=ident, rhs=ident,
                                 start=True, stop=True, skip_group_check=True)
            # q blocks 0,1 + k/v chunk 0
            proj(xq_s[0], 0, 0, pjq, qT, 0, SCALE, bqs_sb)
            proj(xq_s[0], 1, 0, pjq, qT, BLK, SCALE, bqs_sb)
            for sub in range(2):
                proj(xk_s[0], sub, 1, pjq, kT, sub * BLK, None, b_sb[:, 1:2])
            vT_tiles[0] = vT_pool.tile([E, CHUNK], f16, tag="vT", name="vT0")
            for sub in range(2):
                proj(xv_s[0], sub, 2, pjq, vT_tiles[0], sub * BLK,
                     None, b_sb[:, 2:3])

        # ================= pass 1: query blocks 0,1 + streaming proj ====
        with _ES() as p1:
            o_cur = p1.enter_context(tc.tile_pool(name="o1", bufs=1, space="PSUM"))
            s1 = p1.enter_context(tc.tile_pool(name="s1", bufs=2, space="PSUM"))
            pj1 = p1.enter_context(tc.tile_pool(name="pj1", bufs=1, space="PSUM"))
            aux1 = p1.enter_context(tc.tile_pool(name="aux1", bufs=1, space="PSUM"))

            tasks = []
            for c in range(1, NCHUNK):  # project chunk c during chunk c-1
                base = (c - 1) * 8

                def mk(c=c):
                    def k0(): proj(xk_s[c], 0, 1, pj1, kT, c * CHUNK, None, b_sb[:, 1:2])
                    def k1(): proj(xk_s[c], 1, 1, pj1, kT, c * CHUNK + BLK, None, b_sb[:, 1:2])
                    def v0():
                        vT_tiles[c] = vT_pool.tile([E, CHUNK], f16, tag="vT",
                                                   name=f"vT{c}")
                        proj(xv_s[c], 0, 2, pj1, vT_tiles[c], 0, None, b_sb[:, 2:3])
                    def v1(): proj(xv_s[c], 1, 2, pj1, vT_tiles[c], BLK, None, b_sb[:, 2:3])
                    return [k0, k1, v0, v1]
                for off, fn in zip((0, 2, 4, 6), mk()):
                    tasks.append((base + off, fn))
            # q blocks 2,3 (needs xq1) during chunk 1
            tasks.append((9, lambda: proj(xq_s[1], 0, 0, pj1, qT, 2 * BLK,
                                          SCALE, bqs_sb)))
            tasks.append((11, lambda: proj(xq_s[1], 1, 0, pj1, qT, 3 * BLK,
                                           SCALE, bqs_sb)))

            oT01 = attention_pass(s1, aux1, pj1, 0, tasks)
            epilogue(aux1, 0, oT01[0])
            epilogue(aux1, 1, oT01[1])

        # ================= pass 2: query blocks 2,3 =================
        with _ES() as p2:
            o_cur = p2.enter_context(tc.tile_pool(name="o2", bufs=1, space="PSUM"))
            s2p = p2.enter_context(tc.tile_pool(name="s2", bufs=2, space="PSUM"))
            aux2 = p2.enter_context(tc.tile_pool(name="aux2", bufs=1, space="PSUM"))
            oT23 = attention_pass(s2p, None, None, 2, [])
            epilogue(aux2, 2, oT23[0])
            epilogue(aux2, 3, oT23[1])

    nc.finalize()
    return nc


def get_nc():
    if "nc" not in _CACHE:
        _CACHE["nc"] = _build()
    return _CACHE["nc"]


def _feat_major(x2d):
    """[seq, D] fp32 -> [128, CH, seq] fp16 (feature-major, chunked)."""
    xT = np.ascontiguousarray(x2d.T)                 # [D, seq]
    xT = xT.reshape(CH, 128, -1).transpose(1, 0, 2)  # [128, CH, seq]
    return np.ascontiguousarray(xT).astype(np.float16)


def make_in_maps(queries, keys, values, Wq, bq, Wk, bk, Wv, bv):
    def w_prep(w):
        w = np.asarray(w, np.float32).reshape(CH, 128, E)
        return w.transpose(1, 0, 2).astype(np.float16)  # [128, CH, E]

    w_all = np.ascontiguousarray(
        np.stack([w_prep(Wq), w_prep(Wk), w_prep(Wv)], axis=1))
    b_all = np.ascontiguousarray(
        np.stack([bq, bk, bv], axis=1).astype(np.float32))
    shared = {"w_all": w_all, "b_all": b_all}

    queries = np.asarray(queries, np.float32)
    keys = np.asarray(keys, np.float32)
    values = np.asarray(values, np.float32)
    kv_cache = {}
    in_maps = []
    for c in range(NCORES):
        b, h = divmod(c, 2)
        if b not in kv_cache:
            kv_cache[b] = (_feat_major(keys[b]), _feat_major(values[b]))
        xk, xv = kv_cache[b]
        in_maps.append({
            "x_q": _feat_major(queries[b, h * HALF:(h + 1) * HALF, :]),
            "x_k": xk,
            "x_v": xv,
            **shared,
        })
    return in_maps


def run(trace=False, **inputs):
    from concourse.bass_utils import run_bass_kernel_spmd

    nc = get_nc()
    in_maps = make_in_maps(**inputs)
    res = run_bass_kernel_spmd(
        nc, in_maps, core_ids=list(range(NCORES)), trace=trace)
    full = np.empty((B, N, E), dtype=np.float32)
    for c in range(NCORES):
        b, h = divmod(c, 2)
        full[b, h * HALF:(h + 1) * HALF, :] = res.results[c]["out"]
    return full, res


def kernel(**inputs):
    full, _ = run(trace=False, **inputs)
    return full
